# revision 2
# baseline (speedup 1.0000x reference)
"""LSTMCell (B=65536, H=512) Bass/Tile kernel for 8 trn2 NeuronCores — v2.

Data-parallel over batch: each core processes 8192 rows in 4 blocks of 2048.
Weight-stationary ("transposed") orientation: PSUM holds gates as
[128 gate-dims, 2048 batch], so the per-gate-dim bias rides the ACT engine's
free per-partition bias operand and ACT drains PSUM directly with the
activation fused (bias-add costs zero DVE time).

Precision: the three sigmoid gates (f, i, o) run fp8e4 DoubleRow matmuls
(2 contraction elems/cell/cycle); the tanh g-gate — the most error-sensitive
— stays bf16. Weights for fp8 gates are pre-scaled by 4096 on host (keeps
them out of e4m3 subnormals); the 2^-12 descale is folded into ACT's free
scale operand. Measured end-to-end rel err ≈ 1.4e-2 (gate: 2e-2).

Per block (2048 rows), for each of 4 j-slices x 4 gates:
  PE:  fp8 gates: 2 kpair-DR-matmuls x 4 batch-chunks into psum [128,2048]
       g gate:    4 k-matmuls (bf16) x 4 batch-chunks
  ACT: drain = activation(psum*scale + bias[J]) -> bf16 gate tile [128,2048]
Tails per j-slice group (1-2 groups behind): prod=si*tg, c=sf+prod (DVE,
flat 2D bf16 -> 2x mode), tanh(c) (ACT), h=tc*so (GpSimd), out-DMA.
Output layout [512 j, 2, 8192 b] per core; host transposes back to [2,B,H].

Engine budget (per core): ACT ~160us (the wall: 5 transcendental units/elem
at 1 elem/lane/cycle @1.2GHz is a hard floor ~137us), PE ~154us, DVE ~105us,
GpSimd ~76us, DMA ~94us.
"""

import os
import sys

if "/opt/trn_rl_repo" not in sys.path:
    sys.path.insert(0, "/opt/trn_rl_repo")

import numpy as np

import concourse.bacc as bacc
import concourse.mybir as mybir
import concourse.tile as tile

N_CORES = 8
B, H = 65536, 512
B_CORE = B // N_CORES  # 8192
R = 2048  # block rows
NBLK = B_CORE // R  # 4
F32 = mybir.dt.float32
BF16 = mybir.dt.bfloat16
F8 = mybir.dt.float8e4
AF = mybir.ActivationFunctionType
DR = mybir.MatmulPerfMode.DoubleRow
WSCALE = 4096.0  # fp8 weight pre-scale (host); descale via ACT free scale

NEFF_DUMP = "/tmp/lstm_kernel.neff"

GATES = ("f", "i", "o", "g")  # J = gate_idx*4 + jl; g last (tanh, bf16)


def build_module(n_cores=N_CORES):
    nc = bacc.Bacc(
        "TRN2",
        target_bir_lowering=False,
        debug=False,
        num_devices=n_cores,
    )
    x_d = nc.dram_tensor("x", [B_CORE, H], BF16, kind="ExternalInput").ap()
    s_d = nc.dram_tensor("s", [B_CORE, H], BF16, kind="ExternalInput").ap()
    wt8_d = nc.dram_tensor("wt8", [128, 4, 3 * 512], F8, kind="ExternalInput").ap()
    wtg_d = nc.dram_tensor("wtg", [128, 4, 512], BF16, kind="ExternalInput").ap()
    bias_d = nc.dram_tensor("bias", [128, 16], F32, kind="ExternalInput").ap()
    out_d = nc.dram_tensor("out", [H, 2, B_CORE], BF16, kind="ExternalOutput").ap()

    NG = NBLK * 4  # 16 j-slice groups

    with tile.TileContext(nc) as tc:
        with (
            tc.tile_pool(name="const", bufs=1) as cpool,
            tc.tile_pool(name="pxt", bufs=2) as pxt,
            tc.tile_pool(name="pzt", bufs=2) as pzt,
            tc.tile_pool(name="pgate", bufs=2) as pgate,
            tc.tile_pool(name="ptail", bufs=2) as ptail,
            tc.tile_pool(name="pout", bufs=3) as pout,
            tc.tile_pool(name="pg", bufs=1, space="PSUM") as pg,
        ):
            wt8_sb = cpool.tile([128, 4, 3 * 512], F8)
            nc.sync.dma_start(out=wt8_sb[:], in_=wt8_d[:])
            wtg_sb = cpool.tile([128, 4, 512], BF16)
            nc.sync.dma_start(out=wtg_sb[:], in_=wtg_d[:])
            bias_sb = cpool.tile([128, 16], F32)
            nc.sync.dma_start(out=bias_sb[:], in_=bias_d[:])

            xt_t = [None] * NBLK
            st_t = [None] * NBLK
            ztb_t = [None] * NBLK
            zt8_t = [None] * NBLK
            pend = {}  # group -> state for staged tails

            def xbar_quarter(k, q):
                # xt[p, ks, q*512+b] = x[k*2048 + q*512 + b, ks*128 + p]
                if xt_t[k] is None:
                    xt_t[k] = pxt.tile([128, 4, R], BF16, tag="xt", name=f"xt{k}")
                    st_t[k] = pxt.tile([128, 4, R], BF16, tag="st", name=f"st{k}")
                rows = slice(k * R + q * 512, k * R + (q + 1) * 512)
                cols = slice(q * 512, (q + 1) * 512)
                nc.sync.dma_start_transpose(xt_t[k][:, :, cols], x_d[rows, :])
                nc.sync.dma_start_transpose(st_t[k][:, :, cols], s_d[rows, :])

            def zprep(k):
                # ztb = xT + sT (bf16), zt8 = fp8 cast; per-quarter ops
                ztb_t[k] = pzt.tile([128, 4, R], BF16, tag="ztb", name=f"ztb{k}")
                zt8_t[k] = pzt.tile([128, 4, R], F8, tag="zt8", name=f"zt8{k}")
                for q in range(4):
                    cols = slice(q * 512, (q + 1) * 512)
                    nc.vector.tensor_add(
                        ztb_t[k][:, :, cols], xt_t[k][:, :, cols], st_t[k][:, :, cols]
                    )
                    nc.vector.tensor_copy(zt8_t[k][:, :, cols], ztb_t[k][:, :, cols])

            for q in range(4):
                xbar_quarter(0, q)

            for gi in range(NG + 2):
                k, jl = gi // 4, gi % 4

                # tails stage 2 (group gi-2): tanh(c), h = tc*so, out DMA
                if 0 <= gi - 2 < NG:
                    d = pend.pop(gi - 2)
                    tc_t = ptail.tile([128, R], BF16, tag="tc")
                    nc.scalar.activation(tc_t[:], d["coh"][:, 0, :], AF.Tanh)
                    nc.gpsimd.tensor_mul(d["coh"][:, 1, :], tc_t[:], d["o"][:])
                    orows = slice(d["jl"] * 128, (d["jl"] + 1) * 128)
                    ocols = slice(d["k"] * R, (d["k"] + 1) * R)
                    nc.sync.dma_start(out=out_d[orows, :, ocols], in_=d["coh"][:])

                # tails stage 1 (group gi-1): prod = si*tg, c = sf + prod
                if 0 <= gi - 1 < NG:
                    d = pend[gi - 1]
                    prod = ptail.tile([128, R], BF16, tag="prod")
                    nc.vector.tensor_mul(prod[:], d["i"][:], d["g"][:])
                    coh = pout.tile([128, 2, R], BF16, tag="coh")
                    nc.vector.tensor_add(coh[:, 0, :], d["f"][:], prod[:])
                    d["coh"] = coh

                if gi >= NG:
                    continue

                # prefetch one transposed quarter of block k+1 per group
                if k + 1 < NBLK:
                    xbar_quarter(k + 1, jl)

                if jl == 0:
                    zprep(k)

                ztb, zt8 = ztb_t[k], zt8_t[k]
                d = {"k": k, "jl": jl}
                for gate_idx, gname in enumerate(GATES):
                    J = gate_idx * 4 + jl
                    ps = pg.tile(
                        [128, R], F32, tag=f"ps{J % 2}", name=f"ps{gname}{gi}"
                    )
                    if gname != "g":
                        woff = gate_idx * 512 + jl * 128
                        for kp in range(2):
                            for q in range(4):
                                cols = slice(q * 512, (q + 1) * 512)
                                nc.tensor.matmul(
                                    ps[:, cols],
                                    wt8_sb[:, 2 * kp : 2 * kp + 2, woff : woff + 128],
                                    zt8[:, 2 * kp : 2 * kp + 2, cols],
                                    start=(kp == 0),
                                    stop=(kp == 1),
                                    perf_mode=DR,
                                )
                        func, scale = AF.Sigmoid, 1.0 / WSCALE
                    else:
                        for ks in range(4):
                            for q in range(4):
                                cols = slice(q * 512, (q + 1) * 512)
                                nc.tensor.matmul(
                                    ps[:, cols],
                                    wtg_sb[:, ks, jl * 128 : (jl + 1) * 128],
                                    ztb[:, ks, cols],
                                    start=(ks == 0),
                                    stop=(ks == 3),
                                )
                        func, scale = AF.Tanh, 1.0
                    gt = pgate.tile(
                        [128, R],
                        BF16,
                        tag=gname,
                        bufs=3 if gname == "o" else 2,
                        name=f"{gname}{gi}",
                    )
                    nc.scalar.activation(
                        gt[:], ps[:], func, bias=bias_sb[:, J : J + 1], scale=scale
                    )
                    d[gname] = gt
                pend[gi] = d

    nc.compile()
    return nc


def pack_inputs(inputs, short_term_memory, Wf, bf, Wi, bi, Wg, bg, Wo, bo):
    import ml_dtypes

    bf16 = ml_dtypes.bfloat16
    e4 = ml_dtypes.float8_e4m3
    x = np.asarray(inputs, np.float32).astype(bf16)
    s = np.asarray(short_term_memory, np.float32).astype(bf16)
    # fp8 gates f,i,o: wt8[p, ks, gi*512+j] = W[j, ks*128+p] * 4096
    wt8 = np.empty((128, 4, 3 * 512), e4)
    for gidx, W in enumerate([Wf, Wi, Wo]):
        Wt = np.asarray(W, np.float32).T  # [k, j]
        t = np.clip(Wt.reshape(4, 128, 512) * WSCALE, -240.0, 240.0)
        wt8[:, :, gidx * 512 : (gidx + 1) * 512] = t.transpose(1, 0, 2).astype(e4)
    Wgt = np.asarray(Wg, np.float32).T
    wtg = np.ascontiguousarray(
        Wgt.reshape(4, 128, 512).transpose(1, 0, 2).astype(bf16)
    )
    # bias[p, J]: J = gate_idx*4 + jl (gate order f,i,o,g); value b[jl*128+p]
    bias = np.empty((128, 16), np.float32)
    for gidx, b in enumerate([bf, bi, bo, bg]):
        bias[:, gidx * 4 : (gidx + 1) * 4] = (
            np.asarray(b, np.float32).reshape(4, 128).T
        )
    return {"x": x, "s": s, "wt8": wt8, "wtg": wtg, "bias": bias}


class Runner:
    """Compiles the module once and keeps a reusable jitted executor."""

    def __init__(self, nc=None, n_cores=N_CORES):
        import jax
        from concourse import bass2jax as b2j

        self.jax = jax
        self.n_cores = n_cores
        self.nc = nc or build_module(n_cores=n_cores)
        b2j.install_neuronx_cc_hook()

        # dump the final (renamed) NEFF so neuron-profile can pair it with NTFFs
        if not getattr(b2j, "_neff_dump_patched", False):
            orig = b2j.rename_neff_tensors_and_patch_header

            def _patched(neff_path, mapping):
                data = orig(neff_path, mapping)
                with open(NEFF_DUMP, "wb") as f:
                    f.write(data)
                return data

            b2j.rename_neff_tensors_and_patch_header = _patched
            b2j._neff_dump_patched = True

        from jax.experimental.shard_map import shard_map
        from jax.sharding import Mesh, NamedSharding, PartitionSpec

        part_name = (
            self.nc.partition_id_tensor.name if self.nc.partition_id_tensor else None
        )
        in_names, out_names, out_avals = [], [], []
        self.out_shapes = {}
        for alloc in self.nc.m.functions[0].allocations:
            if not isinstance(alloc, mybir.MemoryLocationSet):
                continue
            name = alloc.memorylocations[0].name
            if alloc.kind == "ExternalInput":
                if name != part_name:
                    in_names.append(name)
            elif alloc.kind == "ExternalOutput":
                out_names.append(name)
                shape = tuple(alloc.tensor_shape)
                dt = mybir.dt.np(alloc.dtype)
                out_avals.append(jax.core.ShapedArray(shape, dt))
                self.out_shapes[name] = (shape, dt)
        self.in_names, self.out_names = in_names, out_names
        nc_ref = self.nc

        bind_names = list(in_names) + list(out_names)
        if part_name is not None:
            bind_names.append(part_name)

        def _body(*args):
            operands = list(args)
            if part_name is not None:
                operands.append(b2j.partition_id_tensor())
            outs = b2j._bass_exec_p.bind(
                *operands,
                out_avals=tuple(out_avals),
                in_names=tuple(bind_names),
                out_names=tuple(out_names),
                lowering_input_output_aliases=(),
                sim_require_finite=False,
                sim_require_nnan=False,
                nc=nc_ref,
            )
            return tuple(outs)

        devices = jax.devices()[: self.n_cores]
        mesh = Mesh(np.asarray(devices), ("core",))
        spec = PartitionSpec("core")
        n_args = len(in_names) + len(out_names)
        self.sharding = NamedSharding(mesh, spec)
        self.fn = jax.jit(
            shard_map(
                _body,
                mesh=mesh,
                in_specs=(spec,) * n_args,
                out_specs=(spec,) * len(out_names),
                check_rep=False,
            ),
            keep_unused=True,
        )
        self._dev_args = None

    def stage(self, packed):
        """Transfer inputs (sharded/replicated as needed) to devices once."""
        jax = self.jax
        nc_n = self.n_cores
        args = []
        for name in self.in_names:
            a = packed[name]
            if name in ("x", "s"):
                glob = a  # [B, H]; axis-0 shard = per-core [B_CORE, H]
            else:
                glob = np.concatenate([a] * nc_n, axis=0)  # replicate
            args.append(glob)
        for name in self.out_names:
            shape, dt = self.out_shapes[name]
            args.append(np.zeros((shape[0] * nc_n,) + shape[1:], dt))
        self._dev_args = [jax.device_put(a, self.sharding) for a in args]

    def execute(self):
        outs = self.fn(*self._dev_args)
        self.jax.block_until_ready(outs)
        return outs

    def run(self, packed):
        self.stage(packed)
        outs = self.execute()
        res = {}
        for name, arr in zip(self.out_names, outs):
            a = np.asarray(arr)  # [n_cores*d0, ...]
            shape, _ = self.out_shapes[name]
            res[name] = a.reshape((self.n_cores, shape[0]) + tuple(shape[1:]))
        return res


_RUNNER = None


def _get_runner():
    global _RUNNER
    if _RUNNER is None:
        _RUNNER = Runner()
    return _RUNNER


def kernel(**inputs):
    r = _get_runner()
    packed = pack_inputs(**inputs)
    res = r.run(packed)
    per_core = res["out"]  # [8, 512 j, 2, 8192 b] bf16
    o32 = per_core.astype(np.float32)
    # final[ch, core*8192 + b, j] = o32[core, j, ch, b]
    return np.ascontiguousarray(
        o32.transpose(2, 0, 3, 1).reshape(2, B, H)
    )


if __name__ == "__main__":
    nc = build_module()
    print("module built + compiled OK")


# revision 5
# speedup vs baseline: 1.3969x; 1.3969x over previous
"""LSTMCell (B=65536, H=512) Bass/Tile kernel for 8 trn2 NeuronCores — v2.

Data-parallel over batch: each core processes 8192 rows in 4 blocks of 2048.
Weight-stationary ("transposed") orientation: PSUM holds gates as
[128 gate-dims, 2048 batch], so the per-gate-dim bias rides the ACT engine's
free per-partition bias operand and ACT drains PSUM directly with the
activation fused (bias-add costs zero DVE time).

Precision: the three sigmoid gates (f, i, o) run fp8e4 DoubleRow matmuls
(2 contraction elems/cell/cycle); the tanh g-gate — the most error-sensitive
— stays bf16. Weights for fp8 gates are pre-scaled by 4096 on host (keeps
them out of e4m3 subnormals); the 2^-12 descale is folded into ACT's free
scale operand. Measured end-to-end rel err ≈ 1.4e-2 (gate: 2e-2).

Per block (2048 rows), for each of 4 j-slices x 4 gates:
  PE:  fp8 gates: 2 kpair-DR-matmuls x 4 batch-chunks into psum [128,2048]
       g gate:    4 k-matmuls (bf16) x 4 batch-chunks
  ACT: drain = activation(psum*scale + bias[J]) -> bf16 gate tile [128,2048]
Tails per j-slice group (1-2 groups behind): prod=si*tg, c=sf+prod (DVE,
flat 2D bf16 -> 2x mode), tanh(c) (ACT), h=tc*so (GpSimd), out-DMA.
Output layout [512 j, 2, 8192 b] per core; host transposes back to [2,B,H].

Engine budget (per core): ACT ~160us (the wall: 5 transcendental units/elem
at 1 elem/lane/cycle @1.2GHz is a hard floor ~137us), PE ~154us, DVE ~105us,
GpSimd ~76us, DMA ~94us.
"""

import os
import sys

if "/opt/trn_rl_repo" not in sys.path:
    sys.path.insert(0, "/opt/trn_rl_repo")

import numpy as np

import concourse.bacc as bacc
import concourse.mybir as mybir
import concourse.tile as tile

N_CORES = 8
B, H = 65536, 512
B_CORE = B // N_CORES  # 8192
R = 2048  # block rows
NBLK = B_CORE // R  # 4
F32 = mybir.dt.float32
BF16 = mybir.dt.bfloat16
F8 = mybir.dt.float8e4
AF = mybir.ActivationFunctionType
DR = mybir.MatmulPerfMode.DoubleRow
WSCALE = 4096.0  # fp8 weight pre-scale (host); descale via ACT free scale

NEFF_DUMP = "/tmp/lstm_kernel.neff"

GATES = ("f", "i", "o", "g")  # J = gate_idx*4 + jl; g last (tanh, bf16)


def build_module(n_cores=N_CORES):
    nc = bacc.Bacc(
        "TRN2",
        target_bir_lowering=False,
        debug=False,
        num_devices=n_cores,
    )
    x_d = nc.dram_tensor("x", [B_CORE, H], BF16, kind="ExternalInput").ap()
    s_d = nc.dram_tensor("s", [B_CORE, H], BF16, kind="ExternalInput").ap()
    wt8_d = nc.dram_tensor("wt8", [128, 4, 3 * 512], F8, kind="ExternalInput").ap()
    wtg_d = nc.dram_tensor("wtg", [128, 4, 512], BF16, kind="ExternalInput").ap()
    bias_d = nc.dram_tensor("bias", [128, 16], F32, kind="ExternalInput").ap()
    out_d = nc.dram_tensor("out", [H, 2, B_CORE], BF16, kind="ExternalOutput").ap()

    NG = NBLK * 4  # 16 j-slice groups

    with tile.TileContext(nc) as tc:
        with (
            tc.tile_pool(name="const", bufs=1) as cpool,
            tc.tile_pool(name="pxt", bufs=2) as pxt,
            tc.tile_pool(name="pzt", bufs=2) as pzt,
            tc.tile_pool(name="pgate", bufs=2) as pgate,
            tc.tile_pool(name="ptail", bufs=2) as ptail,
            tc.tile_pool(name="pout", bufs=3) as pout,
            tc.tile_pool(name="pg", bufs=1, space="PSUM") as pg,
        ):
            xt_t = [None] * NBLK
            st_t = [None] * NBLK
            ztb_t = [None] * NBLK
            zt8_t = [None] * NBLK
            pend = {}  # group -> state for staged tails

            def xbar_quarter(k, q, s_eng=None):
                # xt[p, ks, q*512+b] = x[k*2048 + q*512 + b, ks*128 + p]
                if xt_t[k] is None:
                    xt_t[k] = pxt.tile([128, 4, R], BF16, tag="xt", name=f"xt{k}")
                    st_t[k] = pxt.tile([128, 4, R], BF16, tag="st", name=f"st{k}")
                rows = slice(k * R + q * 512, k * R + (q + 1) * 512)
                cols = slice(q * 512, (q + 1) * 512)
                nc.sync.dma_start_transpose(xt_t[k][:, :, cols], x_d[rows, :])
                (s_eng or nc.sync).dma_start_transpose(st_t[k][:, :, cols], s_d[rows, :])

            # prologue: first input quarter before the (larger) weight consts so
            # the z-prep chain starts immediately; block-0 s-transposes ride the
            # idle scalar queue so the sync queue isn't 8 transfers deep
            xbar_quarter(0, 0, s_eng=nc.scalar)
            wt8_sb = cpool.tile([128, 4, 3 * 512], F8)
            nc.sync.dma_start(out=wt8_sb[:], in_=wt8_d[:])
            wtg_sb = cpool.tile([128, 4, 512], BF16)
            nc.sync.dma_start(out=wtg_sb[:], in_=wtg_d[:])
            bias_sb = cpool.tile([128, 16], F32)
            nc.scalar.dma_start(out=bias_sb[:], in_=bias_d[:])

            def zprep(k):
                # ztb = xT + sT (bf16), zt8 = fp8 cast; per-quarter ops
                ztb_t[k] = pzt.tile([128, 4, R], BF16, tag="ztb", name=f"ztb{k}")
                zt8_t[k] = pzt.tile([128, 4, R], F8, tag="zt8", name=f"zt8{k}")
                for q in range(4):
                    cols = slice(q * 512, (q + 1) * 512)
                    nc.vector.tensor_add(
                        ztb_t[k][:, :, cols], xt_t[k][:, :, cols], st_t[k][:, :, cols]
                    )
                    nc.vector.tensor_copy(zt8_t[k][:, :, cols], ztb_t[k][:, :, cols])

            for q in range(1, 4):
                xbar_quarter(0, q, s_eng=nc.scalar)

            for gi in range(NG + 2):
                k, jl = gi // 4, gi % 4

                # tails stage 2 (group gi-2): tanh(c), h = tc*so, out DMA
                if 0 <= gi - 2 < NG:
                    d = pend.pop(gi - 2)
                    tc_t = ptail.tile([128, R], BF16, tag="tc")
                    nc.scalar.activation(tc_t[:], d["coh"][:, 0, :], AF.Tanh)
                    nc.gpsimd.tensor_mul(d["coh"][:, 1, :], tc_t[:], d["o"][:])
                    orows = slice(d["jl"] * 128, (d["jl"] + 1) * 128)
                    ocols = slice(d["k"] * R, (d["k"] + 1) * R)
                    nc.sync.dma_start(out=out_d[orows, :, ocols], in_=d["coh"][:])

                # tails stage 1 (group gi-1): prod = si*tg, c = sf + prod
                if 0 <= gi - 1 < NG:
                    d = pend[gi - 1]
                    prod = ptail.tile([128, R], BF16, tag="prod")
                    nc.vector.tensor_mul(prod[:], d["i"][:], d["g"][:])
                    coh = pout.tile([128, 2, R], BF16, tag="coh")
                    nc.vector.tensor_add(coh[:, 0, :], d["f"][:], prod[:])
                    d["coh"] = coh

                if gi >= NG:
                    continue

                # prefetch one transposed quarter of block k+1 per group
                if k + 1 < NBLK:
                    xbar_quarter(k + 1, jl)

                if jl == 0:
                    zprep(k)

                ztb, zt8 = ztb_t[k], zt8_t[k]
                d = {"k": k, "jl": jl}
                for gate_idx, gname in enumerate(GATES):
                    J = gate_idx * 4 + jl
                    # psum ping-pong must alternate WITHIN a group (gate_idx
                    # parity), not across J (J%2 is constant within a group)
                    ps = pg.tile(
                        [128, R], F32, tag=f"ps{gate_idx % 2}", name=f"ps{gname}{gi}"
                    )
                    if gname != "g":
                        woff = gate_idx * 512 + jl * 128
                        for kp in range(2):
                            for q in range(4):
                                cols = slice(q * 512, (q + 1) * 512)
                                nc.tensor.matmul(
                                    ps[:, cols],
                                    wt8_sb[:, 2 * kp : 2 * kp + 2, woff : woff + 128],
                                    zt8[:, 2 * kp : 2 * kp + 2, cols],
                                    start=(kp == 0),
                                    stop=(kp == 1),
                                    perf_mode=DR,
                                )
                        func, scale = AF.Sigmoid, 1.0 / WSCALE
                    else:
                        for ks in range(4):
                            for q in range(4):
                                cols = slice(q * 512, (q + 1) * 512)
                                nc.tensor.matmul(
                                    ps[:, cols],
                                    wtg_sb[:, ks, jl * 128 : (jl + 1) * 128],
                                    ztb[:, ks, cols],
                                    start=(ks == 0),
                                    stop=(ks == 3),
                                )
                        func, scale = AF.Tanh, 1.0
                    gt = pgate.tile(
                        [128, R],
                        BF16,
                        tag=gname,
                        bufs=3 if gname == "o" else 2,
                        name=f"{gname}{gi}",
                    )
                    nc.scalar.activation(
                        gt[:], ps[:], func, bias=bias_sb[:, J : J + 1], scale=scale
                    )
                    d[gname] = gt
                pend[gi] = d

    nc.compile()
    return nc


def pack_inputs(inputs, short_term_memory, Wf, bf, Wi, bi, Wg, bg, Wo, bo):
    import ml_dtypes

    bf16 = ml_dtypes.bfloat16
    e4 = ml_dtypes.float8_e4m3
    x = np.asarray(inputs, np.float32).astype(bf16)
    s = np.asarray(short_term_memory, np.float32).astype(bf16)
    # fp8 gates f,i,o: wt8[p, ks, gi*512+j] = W[j, ks*128+p] * 4096
    wt8 = np.empty((128, 4, 3 * 512), e4)
    for gidx, W in enumerate([Wf, Wi, Wo]):
        Wt = np.asarray(W, np.float32).T  # [k, j]
        t = np.clip(Wt.reshape(4, 128, 512) * WSCALE, -240.0, 240.0)
        wt8[:, :, gidx * 512 : (gidx + 1) * 512] = t.transpose(1, 0, 2).astype(e4)
    Wgt = np.asarray(Wg, np.float32).T
    wtg = np.ascontiguousarray(
        Wgt.reshape(4, 128, 512).transpose(1, 0, 2).astype(bf16)
    )
    # bias[p, J]: J = gate_idx*4 + jl (gate order f,i,o,g); value b[jl*128+p]
    bias = np.empty((128, 16), np.float32)
    for gidx, b in enumerate([bf, bi, bo, bg]):
        bias[:, gidx * 4 : (gidx + 1) * 4] = (
            np.asarray(b, np.float32).reshape(4, 128).T
        )
    return {"x": x, "s": s, "wt8": wt8, "wtg": wtg, "bias": bias}


class Runner:
    """Compiles the module once and keeps a reusable jitted executor."""

    def __init__(self, nc=None, n_cores=N_CORES):
        import jax
        from concourse import bass2jax as b2j

        self.jax = jax
        self.n_cores = n_cores
        self.nc = nc or build_module(n_cores=n_cores)
        b2j.install_neuronx_cc_hook()

        # dump the final (renamed) NEFF so neuron-profile can pair it with NTFFs
        if not getattr(b2j, "_neff_dump_patched", False):
            orig = b2j.rename_neff_tensors_and_patch_header

            def _patched(neff_path, mapping):
                data = orig(neff_path, mapping)
                with open(NEFF_DUMP, "wb") as f:
                    f.write(data)
                return data

            b2j.rename_neff_tensors_and_patch_header = _patched
            b2j._neff_dump_patched = True

        from jax.experimental.shard_map import shard_map
        from jax.sharding import Mesh, NamedSharding, PartitionSpec

        part_name = (
            self.nc.partition_id_tensor.name if self.nc.partition_id_tensor else None
        )
        in_names, out_names, out_avals = [], [], []
        self.out_shapes = {}
        for alloc in self.nc.m.functions[0].allocations:
            if not isinstance(alloc, mybir.MemoryLocationSet):
                continue
            name = alloc.memorylocations[0].name
            if alloc.kind == "ExternalInput":
                if name != part_name:
                    in_names.append(name)
            elif alloc.kind == "ExternalOutput":
                out_names.append(name)
                shape = tuple(alloc.tensor_shape)
                dt = mybir.dt.np(alloc.dtype)
                out_avals.append(jax.core.ShapedArray(shape, dt))
                self.out_shapes[name] = (shape, dt)
        self.in_names, self.out_names = in_names, out_names
        nc_ref = self.nc

        bind_names = list(in_names) + list(out_names)
        if part_name is not None:
            bind_names.append(part_name)

        def _body(*args):
            operands = list(args)
            if part_name is not None:
                operands.append(b2j.partition_id_tensor())
            outs = b2j._bass_exec_p.bind(
                *operands,
                out_avals=tuple(out_avals),
                in_names=tuple(bind_names),
                out_names=tuple(out_names),
                lowering_input_output_aliases=(),
                sim_require_finite=False,
                sim_require_nnan=False,
                nc=nc_ref,
            )
            return tuple(outs)

        devices = jax.devices()[: self.n_cores]
        mesh = Mesh(np.asarray(devices), ("core",))
        spec = PartitionSpec("core")
        n_args = len(in_names) + len(out_names)
        self.sharding = NamedSharding(mesh, spec)
        self.fn = jax.jit(
            shard_map(
                _body,
                mesh=mesh,
                in_specs=(spec,) * n_args,
                out_specs=(spec,) * len(out_names),
                check_rep=False,
            ),
            keep_unused=True,
        )
        self._dev_args = None

    def stage(self, packed):
        """Transfer inputs (sharded/replicated as needed) to devices once."""
        jax = self.jax
        nc_n = self.n_cores
        args = []
        for name in self.in_names:
            a = packed[name]
            if name in ("x", "s"):
                glob = a  # [B, H]; axis-0 shard = per-core [B_CORE, H]
            else:
                glob = np.concatenate([a] * nc_n, axis=0)  # replicate
            args.append(glob)
        for name in self.out_names:
            shape, dt = self.out_shapes[name]
            args.append(np.zeros((shape[0] * nc_n,) + shape[1:], dt))
        self._dev_args = [jax.device_put(a, self.sharding) for a in args]

    def execute(self):
        outs = self.fn(*self._dev_args)
        self.jax.block_until_ready(outs)
        return outs

    def run(self, packed):
        self.stage(packed)
        outs = self.execute()
        res = {}
        for name, arr in zip(self.out_names, outs):
            a = np.asarray(arr)  # [n_cores*d0, ...]
            shape, _ = self.out_shapes[name]
            res[name] = a.reshape((self.n_cores, shape[0]) + tuple(shape[1:]))
        return res


_RUNNER = None


def _get_runner():
    global _RUNNER
    if _RUNNER is None:
        _RUNNER = Runner()
    return _RUNNER


def kernel(**inputs):
    r = _get_runner()
    packed = pack_inputs(**inputs)
    res = r.run(packed)
    per_core = res["out"]  # [8, 512 j, 2, 8192 b] bf16
    o32 = per_core.astype(np.float32)
    # final[ch, core*8192 + b, j] = o32[core, j, ch, b]
    return np.ascontiguousarray(
        o32.transpose(2, 0, 3, 1).reshape(2, B, H)
    )


if __name__ == "__main__":
    nc = build_module()
    print("module built + compiled OK")


# revision 11
# speedup vs baseline: 1.5315x; 1.0964x over previous
"""LSTMCell (B=65536, H=512) Bass/Tile kernel for 8 trn2 NeuronCores — v2.

Data-parallel over batch: each core processes 8192 rows in 4 blocks of 2048.
Weight-stationary ("transposed") orientation: PSUM holds gates as
[128 gate-dims, 2048 batch], so the per-gate-dim bias rides the ACT engine's
free per-partition bias operand and ACT drains PSUM directly with the
activation fused (bias-add costs zero DVE time).

Precision: the three sigmoid gates (f, i, o) run fp8e4 DoubleRow matmuls
(2 contraction elems/cell/cycle); the tanh g-gate — the most error-sensitive
— stays bf16. Weights for fp8 gates are pre-scaled by 4096 on host (keeps
them out of e4m3 subnormals); the 2^-12 descale is folded into ACT's free
scale operand. Measured end-to-end rel err ≈ 1.4e-2 (gate: 2e-2).

Per block (2048 rows), for each of 4 j-slices x 4 gates:
  PE:  fp8 gates: 2 kpair-DR-matmuls x 4 batch-chunks into psum [128,2048]
       g gate:    4 k-matmuls (bf16) x 4 batch-chunks
  ACT: drain = activation(psum*scale + bias[J]) -> bf16 gate tile [128,2048]
Tails per j-slice group (1-2 groups behind): prod=si*tg, c=sf+prod (DVE,
flat 2D bf16 -> 2x mode), tanh(c) (ACT), h=tc*so (GpSimd), out-DMA.
Output layout [512 j, 2, 8192 b] per core; host transposes back to [2,B,H].

Engine budget (per core): ACT ~160us (the wall: 5 transcendental units/elem
at 1 elem/lane/cycle @1.2GHz is a hard floor ~137us), PE ~154us, DVE ~105us,
GpSimd ~76us, DMA ~94us.
"""

import os
import sys

if "/opt/trn_rl_repo" not in sys.path:
    sys.path.insert(0, "/opt/trn_rl_repo")

import numpy as np

import concourse.bacc as bacc
import concourse.mybir as mybir
import concourse.tile as tile

N_CORES = 8
B, H = 65536, 512
B_CORE = B // N_CORES  # 8192
R = 2048  # block rows
NBLK = B_CORE // R  # 4
F32 = mybir.dt.float32
BF16 = mybir.dt.bfloat16
F8 = mybir.dt.float8e4
AF = mybir.ActivationFunctionType
DR = mybir.MatmulPerfMode.DoubleRow
WSCALE = 4096.0  # fp8 weight pre-scale (host); descale via ACT free scale

NEFF_DUMP = "/tmp/lstm_kernel.neff"

GATES = ("f", "i", "o", "g")  # J = gate_idx*4 + jl; g last (tanh, bf16)


def build_module(n_cores=N_CORES):
    nc = bacc.Bacc(
        "TRN2",
        target_bir_lowering=False,
        debug=False,
        num_devices=n_cores,
    )
    # host-pretransposed: x_d[k, p, ks, b] = x[k*2048 + b, ks*128 + p]
    x_d = nc.dram_tensor("x", [NBLK, 128, 4, R], BF16, kind="ExternalInput").ap()
    s_d = nc.dram_tensor("s", [NBLK, 128, 4, R], BF16, kind="ExternalInput").ap()
    wt8_d = nc.dram_tensor("wt8", [128, 4, 3 * 512], F8, kind="ExternalInput").ap()
    wtg_d = nc.dram_tensor("wtg", [128, 4, 512], BF16, kind="ExternalInput").ap()
    bias_d = nc.dram_tensor("bias", [128, 16], F32, kind="ExternalInput").ap()
    out_d = nc.dram_tensor("out", [H, 2, B_CORE], BF16, kind="ExternalOutput").ap()

    NG = NBLK * 4  # 16 j-slice groups

    with tile.TileContext(nc) as tc:
        with (
            tc.tile_pool(name="const", bufs=1) as cpool,
            tc.tile_pool(name="pxt", bufs=2) as pxt,
            tc.tile_pool(name="pzt", bufs=2) as pzt,
            tc.tile_pool(name="pgate", bufs=2) as pgate,
            tc.tile_pool(name="ptail", bufs=2) as ptail,
            tc.tile_pool(name="pout", bufs=3) as pout,
            tc.tile_pool(name="pg", bufs=1, space="PSUM") as pg,
        ):
            xt_t = [None] * NBLK
            st_t = [None] * NBLK
            ztb_t = [None] * NBLK
            zt8_t = [None] * NBLK
            pend = {}  # group -> state for staged tails

            def fetch_block(k, s_eng=None):
                # plain DMA of host-pretransposed block: [128, 4, 2048] each
                xt_t[k] = pxt.tile([128, 4, R], BF16, tag="xt", name=f"xt{k}")
                st_t[k] = pxt.tile([128, 4, R], BF16, tag="st", name=f"st{k}")
                nc.sync.dma_start(out=xt_t[k][:], in_=x_d[k])
                (s_eng or nc.sync).dma_start(out=st_t[k][:], in_=s_d[k])

            # prologue: inputs for block 0 before the (larger) weight consts so
            # the z-prep chain starts immediately; s rides the idle scalar queue
            fetch_block(0, s_eng=nc.scalar)
            wtg_sb = cpool.tile([128, 4, 512], BF16)
            nc.sync.dma_start(out=wtg_sb[:], in_=wtg_d[:])
            wt8_sb = cpool.tile([128, 4, 3 * 512], F8)
            nc.sync.dma_start(out=wt8_sb[:], in_=wt8_d[:])
            bias_sb = cpool.tile([128, 16], F32)
            nc.scalar.dma_start(out=bias_sb[:], in_=bias_d[:])

            def zprep(k):
                # ztb = xT + sT (bf16), zt8 = fp8 cast. Block 0: per-quarter
                # ops so the first matmuls start ASAP; later blocks: one flat
                # [128, 8192] op each (flat 2D bf16 -> DVE 2x mode).
                ztb_t[k] = pzt.tile([128, 4, R], BF16, tag="ztb", name=f"ztb{k}")
                zt8_t[k] = pzt.tile([128, 4, R], F8, tag="zt8", name=f"zt8{k}")
                ztb_f = ztb_t[k].rearrange("p a b -> p (a b)")
                zt8_f = zt8_t[k].rearrange("p a b -> p (a b)")
                xt_f = xt_t[k].rearrange("p a b -> p (a b)")
                st_f = st_t[k].rearrange("p a b -> p (a b)")
                if k == 0:
                    for q in range(4):
                        cols = slice(q * 2048, (q + 1) * 2048)
                        nc.vector.tensor_add(
                            ztb_f[:, cols], xt_f[:, cols], st_f[:, cols]
                        )
                        nc.vector.tensor_copy(zt8_f[:, cols], ztb_f[:, cols])
                else:
                    nc.vector.tensor_add(ztb_f[:], xt_f[:], st_f[:])
                    nc.vector.tensor_copy(zt8_f[:], ztb_f[:])

            for gi in range(NG + 2):
                k, jl = gi // 4, gi % 4

                # tails stage 2 (group gi-2): tanh(c), h = tc*so, out DMA
                if 0 <= gi - 2 < NG:
                    d = pend.pop(gi - 2)
                    tc_t = ptail.tile([128, R], BF16, tag="tc")
                    nc.scalar.activation(tc_t[:], d["coh"][:, 0, :], AF.Tanh)
                    nc.gpsimd.tensor_mul(d["coh"][:, 1, :], tc_t[:], d["o"][:])
                    orows = slice(d["jl"] * 128, (d["jl"] + 1) * 128)
                    ocols = slice(d["k"] * R, (d["k"] + 1) * R)
                    nc.sync.dma_start(out=out_d[orows, :, ocols], in_=d["coh"][:])

                # tails stage 1 (group gi-1): prod = si*tg, c = sf + prod
                if 0 <= gi - 1 < NG:
                    d = pend[gi - 1]
                    prod = ptail.tile([128, R], BF16, tag="prod")
                    nc.vector.tensor_mul(prod[:], d["i"][:], d["g"][:])
                    coh = pout.tile([128, 2, R], BF16, tag="coh")
                    nc.vector.tensor_add(coh[:, 0, :], d["f"][:], prod[:])
                    d["coh"] = coh

                if gi >= NG:
                    continue

                if gi == 0:
                    zprep(0)
                if k + 1 < NBLK:
                    # fetch next block's inputs early; z-prep one group before
                    # the block boundary so its first (g) matmuls never wait
                    if jl == 0:
                        fetch_block(k + 1)
                    elif jl == 3:
                        zprep(k + 1)

                ztb, zt8 = ztb_t[k], zt8_t[k]
                d = {"k": k, "jl": jl}
                # g first: its 16 bf16 matmuls (2x the fill time of a DR gate)
                # hide under the previous group's trailing drains + tanh(c)
                for slot, gname in enumerate(("g", "f", "i", "o")):
                    gate_idx = GATES.index(gname)
                    J = gate_idx * 4 + jl
                    # psum ping-pong alternates within the group (slot parity)
                    ps = pg.tile(
                        [128, R], F32, tag=f"ps{slot % 2}", name=f"ps{gname}{gi}"
                    )
                    if gname != "g":
                        woff = gate_idx * 512 + jl * 128
                        for kp in range(2):
                            for q in range(4):
                                cols = slice(q * 512, (q + 1) * 512)
                                nc.tensor.matmul(
                                    ps[:, cols],
                                    wt8_sb[:, 2 * kp : 2 * kp + 2, woff : woff + 128],
                                    zt8[:, 2 * kp : 2 * kp + 2, cols],
                                    start=(kp == 0),
                                    stop=(kp == 1),
                                    perf_mode=DR,
                                )
                        func, scale = AF.Sigmoid, 1.0 / WSCALE
                    else:
                        for ks in range(4):
                            for q in range(4):
                                cols = slice(q * 512, (q + 1) * 512)
                                nc.tensor.matmul(
                                    ps[:, cols],
                                    wtg_sb[:, ks, jl * 128 : (jl + 1) * 128],
                                    ztb[:, ks, cols],
                                    start=(ks == 0),
                                    stop=(ks == 3),
                                )
                        func, scale = AF.Tanh, 1.0
                    gt = pgate.tile(
                        [128, R],
                        BF16,
                        tag=gname,
                        bufs=3 if gname == "o" else 2,
                        name=f"{gname}{gi}",
                    )
                    nc.scalar.activation(
                        gt[:], ps[:], func, bias=bias_sb[:, J : J + 1], scale=scale
                    )
                    d[gname] = gt
                pend[gi] = d

    nc.compile()
    return nc


def pack_inputs(inputs, short_term_memory, Wf, bf, Wi, bi, Wg, bg, Wo, bo):
    import ml_dtypes

    bf16 = ml_dtypes.bfloat16
    e4 = ml_dtypes.float8_e4m3

    def pretranspose(a):
        # [B, 512] -> [core*blk, p, ks, b] with a[core*8192 + k*2048 + b,
        # ks*128 + p] at [core*4+k, p, ks, b]
        t = np.asarray(a, np.float32).astype(bf16)
        t = t.reshape(N_CORES, NBLK, R, 4, 128).transpose(0, 1, 4, 3, 2)
        return np.ascontiguousarray(t).reshape(N_CORES * NBLK, 128, 4, R)

    x = pretranspose(inputs)
    s = pretranspose(short_term_memory)
    # fp8 gates f,i,o: wt8[p, ks, gi*512+j] = W[j, ks*128+p] * 4096
    wt8 = np.empty((128, 4, 3 * 512), e4)
    for gidx, W in enumerate([Wf, Wi, Wo]):
        Wt = np.asarray(W, np.float32).T  # [k, j]
        t = np.clip(Wt.reshape(4, 128, 512) * WSCALE, -240.0, 240.0)
        wt8[:, :, gidx * 512 : (gidx + 1) * 512] = t.transpose(1, 0, 2).astype(e4)
    Wgt = np.asarray(Wg, np.float32).T
    wtg = np.ascontiguousarray(
        Wgt.reshape(4, 128, 512).transpose(1, 0, 2).astype(bf16)
    )
    # bias[p, J]: J = gate_idx*4 + jl (gate order f,i,o,g); value b[jl*128+p]
    bias = np.empty((128, 16), np.float32)
    for gidx, b in enumerate([bf, bi, bo, bg]):
        bias[:, gidx * 4 : (gidx + 1) * 4] = (
            np.asarray(b, np.float32).reshape(4, 128).T
        )
    return {"x": x, "s": s, "wt8": wt8, "wtg": wtg, "bias": bias}


class Runner:
    """Compiles the module once and keeps a reusable jitted executor."""

    def __init__(self, nc=None, n_cores=N_CORES):
        import jax
        from concourse import bass2jax as b2j

        self.jax = jax
        self.n_cores = n_cores
        self.nc = nc or build_module(n_cores=n_cores)
        b2j.install_neuronx_cc_hook()

        # dump the final (renamed) NEFF so neuron-profile can pair it with NTFFs
        if not getattr(b2j, "_neff_dump_patched", False):
            orig = b2j.rename_neff_tensors_and_patch_header

            def _patched(neff_path, mapping):
                data = orig(neff_path, mapping)
                with open(NEFF_DUMP, "wb") as f:
                    f.write(data)
                return data

            b2j.rename_neff_tensors_and_patch_header = _patched
            b2j._neff_dump_patched = True

        from jax.experimental.shard_map import shard_map
        from jax.sharding import Mesh, NamedSharding, PartitionSpec

        part_name = (
            self.nc.partition_id_tensor.name if self.nc.partition_id_tensor else None
        )
        in_names, out_names, out_avals = [], [], []
        self.out_shapes = {}
        for alloc in self.nc.m.functions[0].allocations:
            if not isinstance(alloc, mybir.MemoryLocationSet):
                continue
            name = alloc.memorylocations[0].name
            if alloc.kind == "ExternalInput":
                if name != part_name:
                    in_names.append(name)
            elif alloc.kind == "ExternalOutput":
                out_names.append(name)
                shape = tuple(alloc.tensor_shape)
                dt = mybir.dt.np(alloc.dtype)
                out_avals.append(jax.core.ShapedArray(shape, dt))
                self.out_shapes[name] = (shape, dt)
        self.in_names, self.out_names = in_names, out_names
        nc_ref = self.nc

        bind_names = list(in_names) + list(out_names)
        if part_name is not None:
            bind_names.append(part_name)

        def _body(*args):
            operands = list(args)
            if part_name is not None:
                operands.append(b2j.partition_id_tensor())
            outs = b2j._bass_exec_p.bind(
                *operands,
                out_avals=tuple(out_avals),
                in_names=tuple(bind_names),
                out_names=tuple(out_names),
                lowering_input_output_aliases=(),
                sim_require_finite=False,
                sim_require_nnan=False,
                nc=nc_ref,
            )
            return tuple(outs)

        devices = jax.devices()[: self.n_cores]
        mesh = Mesh(np.asarray(devices), ("core",))
        spec = PartitionSpec("core")
        n_args = len(in_names) + len(out_names)
        self.sharding = NamedSharding(mesh, spec)
        self.fn = jax.jit(
            shard_map(
                _body,
                mesh=mesh,
                in_specs=(spec,) * n_args,
                out_specs=(spec,) * len(out_names),
                check_rep=False,
            ),
            keep_unused=True,
        )
        self._dev_args = None

    def stage(self, packed):
        """Transfer inputs (sharded/replicated as needed) to devices once."""
        jax = self.jax
        nc_n = self.n_cores
        args = []
        for name in self.in_names:
            a = packed[name]
            if name in ("x", "s"):
                glob = a  # [B, H]; axis-0 shard = per-core [B_CORE, H]
            else:
                glob = np.concatenate([a] * nc_n, axis=0)  # replicate
            args.append(glob)
        for name in self.out_names:
            shape, dt = self.out_shapes[name]
            args.append(np.zeros((shape[0] * nc_n,) + shape[1:], dt))
        self._dev_args = [jax.device_put(a, self.sharding) for a in args]

    def execute(self):
        outs = self.fn(*self._dev_args)
        self.jax.block_until_ready(outs)
        return outs

    def run(self, packed):
        self.stage(packed)
        outs = self.execute()
        res = {}
        for name, arr in zip(self.out_names, outs):
            a = np.asarray(arr)  # [n_cores*d0, ...]
            shape, _ = self.out_shapes[name]
            res[name] = a.reshape((self.n_cores, shape[0]) + tuple(shape[1:]))
        return res


_RUNNER = None


def _get_runner():
    global _RUNNER
    if _RUNNER is None:
        _RUNNER = Runner()
    return _RUNNER


def kernel(**inputs):
    r = _get_runner()
    packed = pack_inputs(**inputs)
    res = r.run(packed)
    per_core = res["out"]  # [8, 512 j, 2, 8192 b] bf16
    o32 = per_core.astype(np.float32)
    # final[ch, core*8192 + b, j] = o32[core, j, ch, b]
    return np.ascontiguousarray(
        o32.transpose(2, 0, 3, 1).reshape(2, B, H)
    )


if __name__ == "__main__":
    nc = build_module()
    print("module built + compiled OK")


# revision 15
# speedup vs baseline: 1.5566x; 1.0163x over previous
"""LSTMCell (B=65536, H=512) Bass/Tile kernel for 8 trn2 NeuronCores — v2.

Data-parallel over batch: each core processes 8192 rows in 4 blocks of 2048.
Weight-stationary ("transposed") orientation: PSUM holds gates as
[128 gate-dims, 2048 batch], so the per-gate-dim bias rides the ACT engine's
free per-partition bias operand and ACT drains PSUM directly with the
activation fused (bias-add costs zero DVE time).

Precision: the three sigmoid gates (f, i, o) run fp8e4 DoubleRow matmuls
(2 contraction elems/cell/cycle); the tanh g-gate — the most error-sensitive
— stays bf16. Weights for fp8 gates are pre-scaled by 4096 on host (keeps
them out of e4m3 subnormals); the 2^-12 descale is folded into ACT's free
scale operand. Measured end-to-end rel err ≈ 1.4e-2 (gate: 2e-2).

Per block (2048 rows), for each of 4 j-slices x 4 gates:
  PE:  fp8 gates: 2 kpair-DR-matmuls x 4 batch-chunks into psum [128,2048]
       g gate:    4 k-matmuls (bf16) x 4 batch-chunks
  ACT: drain = activation(psum*scale + bias[J]) -> bf16 gate tile [128,2048]
Tails per j-slice group (1-2 groups behind): prod=si*tg, c=sf+prod (DVE,
flat 2D bf16 -> 2x mode), tanh(c) (ACT), h=tc*so (GpSimd), out-DMA.
Output layout [512 j, 2, 8192 b] per core; host transposes back to [2,B,H].

Engine budget (per core): ACT ~160us (the wall: 5 transcendental units/elem
at 1 elem/lane/cycle @1.2GHz is a hard floor ~137us), PE ~154us, DVE ~105us,
GpSimd ~76us, DMA ~94us.
"""

import os
import sys

if "/opt/trn_rl_repo" not in sys.path:
    sys.path.insert(0, "/opt/trn_rl_repo")

import numpy as np

import concourse.bacc as bacc
import concourse.mybir as mybir
import concourse.tile as tile

N_CORES = 8
B, H = 65536, 512
B_CORE = B // N_CORES  # 8192
R = 2048  # block rows
NBLK = B_CORE // R  # 4
F32 = mybir.dt.float32
BF16 = mybir.dt.bfloat16
F8 = mybir.dt.float8e4
AF = mybir.ActivationFunctionType
DR = mybir.MatmulPerfMode.DoubleRow
WSCALE = 4096.0  # fp8 weight pre-scale (host); descale via ACT free scale

NEFF_DUMP = "/tmp/lstm_kernel.neff"

GATES = ("f", "i", "o", "g")  # J = gate_idx*4 + jl; g last (tanh, bf16)


def build_module(n_cores=N_CORES):
    nc = bacc.Bacc(
        "TRN2",
        target_bir_lowering=False,
        debug=False,
        num_devices=n_cores,
    )
    # host-pretransposed: x_d[k, p, ks, b] = x[k*2048 + b, ks*128 + p]
    x_d = nc.dram_tensor("x", [NBLK, 128, 4, R], BF16, kind="ExternalInput").ap()
    s_d = nc.dram_tensor("s", [NBLK, 128, 4, R], BF16, kind="ExternalInput").ap()
    wt8_d = nc.dram_tensor("wt8", [128, 4, 3 * 512], F8, kind="ExternalInput").ap()
    wtg_d = nc.dram_tensor("wtg", [128, 4, 512], BF16, kind="ExternalInput").ap()
    bias_d = nc.dram_tensor("bias", [128, 16], F32, kind="ExternalInput").ap()
    out_d = nc.dram_tensor("out", [H, 2, B_CORE], BF16, kind="ExternalOutput").ap()

    NG = NBLK * 4  # 16 j-slice groups

    with tile.TileContext(nc) as tc:
        with (
            tc.tile_pool(name="const", bufs=1) as cpool,
            tc.tile_pool(name="pxt", bufs=2) as pxt,
            tc.tile_pool(name="pzt", bufs=2) as pzt,
            tc.tile_pool(name="pgate", bufs=2) as pgate,
            tc.tile_pool(name="ptail", bufs=2) as ptail,
            tc.tile_pool(name="pout", bufs=4) as pout,
            tc.tile_pool(name="pg", bufs=1, space="PSUM") as pg,
        ):
            xt_t = [None] * NBLK
            st_t = [None] * NBLK
            ztb_t = [None] * NBLK
            zt8_t = [None] * NBLK
            pend = {}  # group -> state for staged tails

            def fetch_block(k, s_eng=None, by_ks=False):
                # plain DMA of host-pretransposed block: [128, 4, 2048] each
                xt_t[k] = pxt.tile([128, 4, R], BF16, tag="xt", name=f"xt{k}")
                st_t[k] = pxt.tile([128, 4, R], BF16, tag="st", name=f"st{k}")
                if by_ks:
                    # k-slice granular so block-0's z-prep chain starts after
                    # ~1.3us instead of after the whole 2MB transfer
                    for ks in range(4):
                        nc.sync.dma_start(out=xt_t[k][:, ks, :], in_=x_d[k][:, ks, :])
                        (s_eng or nc.sync).dma_start(
                            out=st_t[k][:, ks, :], in_=s_d[k][:, ks, :]
                        )
                else:
                    nc.sync.dma_start(out=xt_t[k][:], in_=x_d[k])
                    (s_eng or nc.sync).dma_start(out=st_t[k][:], in_=s_d[k])

            # prologue: inputs for block 0 before the (larger) weight consts so
            # the z-prep chain starts immediately; s rides the idle scalar queue
            fetch_block(0, s_eng=nc.scalar, by_ks=True)
            wtg_sb = cpool.tile([128, 4, 512], BF16)
            nc.sync.dma_start(out=wtg_sb[:], in_=wtg_d[:])
            wt8_sb = cpool.tile([128, 4, 3 * 512], F8)
            nc.sync.dma_start(out=wt8_sb[:], in_=wt8_d[:])
            bias_sb = cpool.tile([128, 16], F32)
            nc.scalar.dma_start(out=bias_sb[:], in_=bias_d[:])

            def zprep(k):
                # ztb = xT + sT (bf16), zt8 = fp8 cast. Block 0: per-quarter
                # ops so the first matmuls start ASAP; later blocks: one flat
                # [128, 8192] op each (flat 2D bf16 -> DVE 2x mode).
                ztb_t[k] = pzt.tile([128, 4, R], BF16, tag="ztb", name=f"ztb{k}")
                zt8_t[k] = pzt.tile([128, 4, R], F8, tag="zt8", name=f"zt8{k}")
                ztb_f = ztb_t[k].rearrange("p a b -> p (a b)")
                zt8_f = zt8_t[k].rearrange("p a b -> p (a b)")
                xt_f = xt_t[k].rearrange("p a b -> p (a b)")
                st_f = st_t[k].rearrange("p a b -> p (a b)")
                if k == 0:
                    # all bf16 adds first (g-gate matmuls consume them ks by
                    # ks), fp8 casts after
                    for q in range(4):
                        cols = slice(q * 2048, (q + 1) * 2048)
                        nc.vector.tensor_add(
                            ztb_f[:, cols], xt_f[:, cols], st_f[:, cols]
                        )
                    for q in range(4):
                        cols = slice(q * 2048, (q + 1) * 2048)
                        nc.vector.tensor_copy(zt8_f[:, cols], ztb_f[:, cols])
                else:
                    nc.vector.tensor_add(ztb_f[:], xt_f[:], st_f[:])
                    nc.vector.tensor_copy(zt8_f[:], ztb_f[:])

            for gi in range(NG + 2):
                k, jl = gi // 4, gi % 4

                # tails stage 2 (group gi-2): tanh(c), h = tc*so, out DMA
                if 0 <= gi - 2 < NG:
                    d = pend.pop(gi - 2)
                    tc_t = ptail.tile([128, R], BF16, tag="tc")
                    nc.scalar.activation(tc_t[:], d["coh"][:, 0, :], AF.Tanh)
                    nc.gpsimd.tensor_mul(d["coh"][:, 1, :], tc_t[:], d["o"][:])
                    orows = slice(d["jl"] * 128, (d["jl"] + 1) * 128)
                    ocols = slice(d["k"] * R, (d["k"] + 1) * R)
                    nc.sync.dma_start(out=out_d[orows, :, ocols], in_=d["coh"][:])

                # tails stage 1 (group gi-1): prod = si*tg, c = sf + prod
                if 0 <= gi - 1 < NG:
                    d = pend[gi - 1]
                    prod = ptail.tile([128, R], BF16, tag="prod")
                    nc.vector.tensor_mul(prod[:], d["i"][:], d["g"][:])
                    coh = pout.tile([128, 2, R], BF16, tag="coh")
                    nc.vector.tensor_add(coh[:, 0, :], d["f"][:], prod[:])
                    d["coh"] = coh

                if gi >= NG:
                    continue

                if gi == 0:
                    zprep(0)
                if k + 1 < NBLK:
                    # fetch next block's inputs early; z-prep one group before
                    # the block boundary so its first (g) matmuls never wait
                    if jl == 0:
                        fetch_block(k + 1)
                    elif jl == 2:
                        zprep(k + 1)

                ztb, zt8 = ztb_t[k], zt8_t[k]
                d = {"k": k, "jl": jl}
                # g first: its 16 bf16 matmuls (2x the fill time of a DR gate)
                # hide under the previous group's trailing drains + tanh(c)
                for slot, gname in enumerate(("g", "f", "i", "o")):
                    gate_idx = GATES.index(gname)
                    J = gate_idx * 4 + jl
                    # psum ping-pong alternates within the group (slot parity)
                    ps = pg.tile(
                        [128, R], F32, tag=f"ps{slot % 2}", name=f"ps{gname}{gi}"
                    )
                    if gname != "g":
                        woff = gate_idx * 512 + jl * 128
                        for kp in range(2):
                            for q in range(4):
                                cols = slice(q * 512, (q + 1) * 512)
                                nc.tensor.matmul(
                                    ps[:, cols],
                                    wt8_sb[:, 2 * kp : 2 * kp + 2, woff : woff + 128],
                                    zt8[:, 2 * kp : 2 * kp + 2, cols],
                                    start=(kp == 0),
                                    stop=(kp == 1),
                                    perf_mode=DR,
                                )
                        func, scale = AF.Sigmoid, 1.0 / WSCALE
                    else:
                        for ks in range(4):
                            for q in range(4):
                                cols = slice(q * 512, (q + 1) * 512)
                                nc.tensor.matmul(
                                    ps[:, cols],
                                    wtg_sb[:, ks, jl * 128 : (jl + 1) * 128],
                                    ztb[:, ks, cols],
                                    start=(ks == 0),
                                    stop=(ks == 3),
                                )
                        func, scale = AF.Tanh, 1.0
                    gt = pgate.tile(
                        [128, R],
                        BF16,
                        tag=gname,
                        bufs=3 if gname == "o" else 2,
                        name=f"{gname}{gi}",
                    )
                    nc.scalar.activation(
                        gt[:], ps[:], func, bias=bias_sb[:, J : J + 1], scale=scale
                    )
                    d[gname] = gt
                pend[gi] = d

    nc.compile()
    return nc


def pack_inputs(inputs, short_term_memory, Wf, bf, Wi, bi, Wg, bg, Wo, bo):
    import ml_dtypes

    bf16 = ml_dtypes.bfloat16
    e4 = ml_dtypes.float8_e4m3

    def pretranspose(a):
        # [B, 512] -> [core*blk, p, ks, b] with a[core*8192 + k*2048 + b,
        # ks*128 + p] at [core*4+k, p, ks, b]
        t = np.asarray(a, np.float32).astype(bf16)
        t = t.reshape(N_CORES, NBLK, R, 4, 128).transpose(0, 1, 4, 3, 2)
        return np.ascontiguousarray(t).reshape(N_CORES * NBLK, 128, 4, R)

    x = pretranspose(inputs)
    s = pretranspose(short_term_memory)
    # fp8 gates f,i,o: wt8[p, ks, gi*512+j] = W[j, ks*128+p] * 4096
    wt8 = np.empty((128, 4, 3 * 512), e4)
    for gidx, W in enumerate([Wf, Wi, Wo]):
        Wt = np.asarray(W, np.float32).T  # [k, j]
        t = np.clip(Wt.reshape(4, 128, 512) * WSCALE, -240.0, 240.0)
        wt8[:, :, gidx * 512 : (gidx + 1) * 512] = t.transpose(1, 0, 2).astype(e4)
    Wgt = np.asarray(Wg, np.float32).T
    wtg = np.ascontiguousarray(
        Wgt.reshape(4, 128, 512).transpose(1, 0, 2).astype(bf16)
    )
    # bias[p, J]: J = gate_idx*4 + jl (gate order f,i,o,g); value b[jl*128+p]
    bias = np.empty((128, 16), np.float32)
    for gidx, b in enumerate([bf, bi, bo, bg]):
        bias[:, gidx * 4 : (gidx + 1) * 4] = (
            np.asarray(b, np.float32).reshape(4, 128).T
        )
    return {"x": x, "s": s, "wt8": wt8, "wtg": wtg, "bias": bias}


class Runner:
    """Compiles the module once and keeps a reusable jitted executor."""

    def __init__(self, nc=None, n_cores=N_CORES):
        import jax
        from concourse import bass2jax as b2j

        self.jax = jax
        self.n_cores = n_cores
        self.nc = nc or build_module(n_cores=n_cores)
        b2j.install_neuronx_cc_hook()

        # dump the final (renamed) NEFF so neuron-profile can pair it with NTFFs
        if not getattr(b2j, "_neff_dump_patched", False):
            orig = b2j.rename_neff_tensors_and_patch_header

            def _patched(neff_path, mapping):
                data = orig(neff_path, mapping)
                with open(NEFF_DUMP, "wb") as f:
                    f.write(data)
                return data

            b2j.rename_neff_tensors_and_patch_header = _patched
            b2j._neff_dump_patched = True

        from jax.experimental.shard_map import shard_map
        from jax.sharding import Mesh, NamedSharding, PartitionSpec

        part_name = (
            self.nc.partition_id_tensor.name if self.nc.partition_id_tensor else None
        )
        in_names, out_names, out_avals = [], [], []
        self.out_shapes = {}
        for alloc in self.nc.m.functions[0].allocations:
            if not isinstance(alloc, mybir.MemoryLocationSet):
                continue
            name = alloc.memorylocations[0].name
            if alloc.kind == "ExternalInput":
                if name != part_name:
                    in_names.append(name)
            elif alloc.kind == "ExternalOutput":
                out_names.append(name)
                shape = tuple(alloc.tensor_shape)
                dt = mybir.dt.np(alloc.dtype)
                out_avals.append(jax.core.ShapedArray(shape, dt))
                self.out_shapes[name] = (shape, dt)
        self.in_names, self.out_names = in_names, out_names
        nc_ref = self.nc

        bind_names = list(in_names) + list(out_names)
        if part_name is not None:
            bind_names.append(part_name)

        def _body(*args):
            operands = list(args)
            if part_name is not None:
                operands.append(b2j.partition_id_tensor())
            outs = b2j._bass_exec_p.bind(
                *operands,
                out_avals=tuple(out_avals),
                in_names=tuple(bind_names),
                out_names=tuple(out_names),
                lowering_input_output_aliases=(),
                sim_require_finite=False,
                sim_require_nnan=False,
                nc=nc_ref,
            )
            return tuple(outs)

        devices = jax.devices()[: self.n_cores]
        mesh = Mesh(np.asarray(devices), ("core",))
        spec = PartitionSpec("core")
        n_args = len(in_names) + len(out_names)
        self.sharding = NamedSharding(mesh, spec)
        self.fn = jax.jit(
            shard_map(
                _body,
                mesh=mesh,
                in_specs=(spec,) * n_args,
                out_specs=(spec,) * len(out_names),
                check_rep=False,
            ),
            keep_unused=True,
        )
        self._dev_args = None

    def stage(self, packed):
        """Transfer inputs (sharded/replicated as needed) to devices once."""
        jax = self.jax
        nc_n = self.n_cores
        args = []
        for name in self.in_names:
            a = packed[name]
            if name in ("x", "s"):
                glob = a  # [B, H]; axis-0 shard = per-core [B_CORE, H]
            else:
                glob = np.concatenate([a] * nc_n, axis=0)  # replicate
            args.append(glob)
        for name in self.out_names:
            shape, dt = self.out_shapes[name]
            args.append(np.zeros((shape[0] * nc_n,) + shape[1:], dt))
        self._dev_args = [jax.device_put(a, self.sharding) for a in args]

    def execute(self):
        outs = self.fn(*self._dev_args)
        self.jax.block_until_ready(outs)
        return outs

    def run(self, packed):
        self.stage(packed)
        outs = self.execute()
        res = {}
        for name, arr in zip(self.out_names, outs):
            a = np.asarray(arr)  # [n_cores*d0, ...]
            shape, _ = self.out_shapes[name]
            res[name] = a.reshape((self.n_cores, shape[0]) + tuple(shape[1:]))
        return res


_RUNNER = None


def _get_runner():
    global _RUNNER
    if _RUNNER is None:
        _RUNNER = Runner()
    return _RUNNER


def kernel(**inputs):
    r = _get_runner()
    packed = pack_inputs(**inputs)
    res = r.run(packed)
    per_core = res["out"]  # [8, 512 j, 2, 8192 b] bf16
    o32 = per_core.astype(np.float32)
    # final[ch, core*8192 + b, j] = o32[core, j, ch, b]
    return np.ascontiguousarray(
        o32.transpose(2, 0, 3, 1).reshape(2, B, H)
    )


if __name__ == "__main__":
    nc = build_module()
    print("module built + compiled OK")


# revision 16
# speedup vs baseline: 1.5823x; 1.0166x over previous
"""LSTMCell (B=65536, H=512) Bass/Tile kernel for 8 trn2 NeuronCores — v2.

Data-parallel over batch: each core processes 8192 rows in 4 blocks of 2048.
Weight-stationary ("transposed") orientation: PSUM holds gates as
[128 gate-dims, 2048 batch], so the per-gate-dim bias rides the ACT engine's
free per-partition bias operand and ACT drains PSUM directly with the
activation fused (bias-add costs zero DVE time).

Precision: the three sigmoid gates (f, i, o) run fp8e4 DoubleRow matmuls
(2 contraction elems/cell/cycle); the tanh g-gate — the most error-sensitive
— stays bf16. Weights for fp8 gates are pre-scaled by 4096 on host (keeps
them out of e4m3 subnormals); the 2^-12 descale is folded into ACT's free
scale operand. Measured end-to-end rel err ≈ 1.4e-2 (gate: 2e-2).

Per block (2048 rows), for each of 4 j-slices x 4 gates:
  PE:  fp8 gates: 2 kpair-DR-matmuls x 4 batch-chunks into psum [128,2048]
       g gate:    4 k-matmuls (bf16) x 4 batch-chunks
  ACT: drain = activation(psum*scale + bias[J]) -> bf16 gate tile [128,2048]
Tails per j-slice group (1-2 groups behind): prod=si*tg, c=sf+prod (DVE,
flat 2D bf16 -> 2x mode), tanh(c) (ACT), h=tc*so (GpSimd), out-DMA.
Output layout [512 j, 2, 8192 b] per core; host transposes back to [2,B,H].

Engine budget (per core): ACT ~160us (the wall: 5 transcendental units/elem
at 1 elem/lane/cycle @1.2GHz is a hard floor ~137us), PE ~154us, DVE ~105us,
GpSimd ~76us, DMA ~94us.
"""

import os
import sys

if "/opt/trn_rl_repo" not in sys.path:
    sys.path.insert(0, "/opt/trn_rl_repo")

import numpy as np

import concourse.bacc as bacc
import concourse.mybir as mybir
import concourse.tile as tile

N_CORES = 8
B, H = 65536, 512
B_CORE = B // N_CORES  # 8192
R = 2048  # block rows
NBLK = B_CORE // R  # 4
F32 = mybir.dt.float32
BF16 = mybir.dt.bfloat16
F8 = mybir.dt.float8e4
AF = mybir.ActivationFunctionType
DR = mybir.MatmulPerfMode.DoubleRow
WSCALE = 4096.0  # fp8 weight pre-scale (host); descale via ACT free scale

NEFF_DUMP = "/tmp/lstm_kernel.neff"

GATES = ("f", "i", "o", "g")  # J = gate_idx*4 + jl; g last (tanh, bf16)


def build_module(n_cores=N_CORES):
    nc = bacc.Bacc(
        "TRN2",
        target_bir_lowering=False,
        debug=False,
        num_devices=n_cores,
    )
    # host-pretransposed: x_d[k, p, ks, b] = x[k*2048 + b, ks*128 + p]
    x_d = nc.dram_tensor("x", [NBLK, 128, 4, R], BF16, kind="ExternalInput").ap()
    s_d = nc.dram_tensor("s", [NBLK, 128, 4, R], BF16, kind="ExternalInput").ap()
    wt8_d = nc.dram_tensor("wt8", [128, 4, 3 * 512], F8, kind="ExternalInput").ap()
    wtg_d = nc.dram_tensor("wtg", [128, 4, 512], BF16, kind="ExternalInput").ap()
    bias_d = nc.dram_tensor("bias", [128, 16], F32, kind="ExternalInput").ap()
    out_d = nc.dram_tensor("out", [H, 2, B_CORE], BF16, kind="ExternalOutput").ap()

    NG = NBLK * 4  # 16 j-slice groups

    with tile.TileContext(nc) as tc:
        with (
            tc.tile_pool(name="const", bufs=1) as cpool,
            tc.tile_pool(name="pxt", bufs=2) as pxt,
            tc.tile_pool(name="pzt", bufs=2) as pzt,
            tc.tile_pool(name="pgate", bufs=2) as pgate,
            tc.tile_pool(name="ptail", bufs=2) as ptail,
            tc.tile_pool(name="pout", bufs=4) as pout,
            tc.tile_pool(name="pg", bufs=1, space="PSUM") as pg,
        ):
            xt_t = [None] * NBLK
            st_t = [None] * NBLK
            ztb_t = [None] * NBLK
            zt8_t = [None] * NBLK
            pend = {}  # group -> state for staged tails

            def fetch_block(k, s_eng=None, by_ks=False):
                # plain DMA of host-pretransposed block: [128, 4, 2048] each
                xt_t[k] = pxt.tile([128, 4, R], BF16, tag="xt", name=f"xt{k}")
                st_t[k] = pxt.tile([128, 4, R], BF16, tag="st", name=f"st{k}")
                if by_ks:
                    # k-slice granular so block-0's z-prep chain starts after
                    # ~1.3us instead of after the whole 2MB transfer
                    for ks in range(4):
                        nc.sync.dma_start(out=xt_t[k][:, ks, :], in_=x_d[k][:, ks, :])
                        (s_eng or nc.sync).dma_start(
                            out=st_t[k][:, ks, :], in_=s_d[k][:, ks, :]
                        )
                else:
                    nc.sync.dma_start(out=xt_t[k][:], in_=x_d[k])
                    (s_eng or nc.sync).dma_start(out=st_t[k][:], in_=s_d[k])

            # prologue: inputs for block 0 before the (larger) weight consts so
            # the z-prep chain starts immediately; s rides the idle scalar queue
            fetch_block(0, s_eng=nc.scalar, by_ks=True)
            wtg_sb = cpool.tile([128, 4, 512], BF16)
            nc.sync.dma_start(out=wtg_sb[:], in_=wtg_d[:])
            wt8_sb = cpool.tile([128, 4, 3 * 512], F8)
            nc.sync.dma_start(out=wt8_sb[:], in_=wt8_d[:])
            bias_sb = cpool.tile([128, 16], F32)
            nc.scalar.dma_start(out=bias_sb[:], in_=bias_d[:])

            def zprep(k):
                # ztb = xT + sT (bf16), zt8 = fp8 cast. Block 0: per-quarter
                # ops so the first matmuls start ASAP; later blocks: one flat
                # [128, 8192] op each (flat 2D bf16 -> DVE 2x mode).
                ztb_t[k] = pzt.tile([128, 4, R], BF16, tag="ztb", name=f"ztb{k}")
                zt8_t[k] = pzt.tile([128, 4, R], F8, tag="zt8", name=f"zt8{k}")
                if k == 0:
                    # ks-granular (the 3D slices the matmuls read): bf16 adds
                    # first (g-gate consumes them ks by ks), fp8 casts after
                    for q in range(4):
                        nc.vector.tensor_add(
                            ztb_t[k][:, q, :], xt_t[k][:, q, :], st_t[k][:, q, :]
                        )
                    for q in range(4):
                        nc.vector.tensor_copy(zt8_t[k][:, q, :], ztb_t[k][:, q, :])
                else:
                    ztb_f = ztb_t[k].rearrange("p a b -> p (a b)")
                    zt8_f = zt8_t[k].rearrange("p a b -> p (a b)")
                    xt_f = xt_t[k].rearrange("p a b -> p (a b)")
                    st_f = st_t[k].rearrange("p a b -> p (a b)")
                    nc.vector.tensor_add(ztb_f[:], xt_f[:], st_f[:])
                    nc.vector.tensor_copy(zt8_f[:], ztb_f[:])

            for gi in range(NG + 2):
                k, jl = gi // 4, gi % 4

                # tails stage 2 (group gi-2): tanh(c), h = tc*so, out DMA
                if 0 <= gi - 2 < NG:
                    d = pend.pop(gi - 2)
                    tc_t = ptail.tile([128, R], BF16, tag="tc")
                    nc.scalar.activation(tc_t[:], d["coh"][:, 0, :], AF.Tanh)
                    nc.gpsimd.tensor_mul(d["coh"][:, 1, :], tc_t[:], d["o"][:])
                    orows = slice(d["jl"] * 128, (d["jl"] + 1) * 128)
                    ocols = slice(d["k"] * R, (d["k"] + 1) * R)
                    nc.sync.dma_start(out=out_d[orows, :, ocols], in_=d["coh"][:])

                # tails stage 1 (group gi-1): prod = si*tg, c = sf + prod
                if 0 <= gi - 1 < NG:
                    d = pend[gi - 1]
                    prod = ptail.tile([128, R], BF16, tag="prod")
                    nc.vector.tensor_mul(prod[:], d["i"][:], d["g"][:])
                    coh = pout.tile([128, 2, R], BF16, tag="coh")
                    nc.vector.tensor_add(coh[:, 0, :], d["f"][:], prod[:])
                    d["coh"] = coh

                if gi >= NG:
                    continue

                if gi == 0:
                    zprep(0)
                if k + 1 < NBLK:
                    # fetch next block's inputs early; z-prep one group before
                    # the block boundary so its first (g) matmuls never wait
                    if jl == 0:
                        fetch_block(k + 1)
                    elif jl == 2:
                        zprep(k + 1)

                ztb, zt8 = ztb_t[k], zt8_t[k]
                d = {"k": k, "jl": jl}
                # g first: its 16 bf16 matmuls (2x the fill time of a DR gate)
                # hide under the previous group's trailing drains + tanh(c)
                for slot, gname in enumerate(("g", "f", "i", "o")):
                    gate_idx = GATES.index(gname)
                    J = gate_idx * 4 + jl
                    # psum ping-pong alternates within the group (slot parity)
                    ps = pg.tile(
                        [128, R], F32, tag=f"ps{slot % 2}", name=f"ps{gname}{gi}"
                    )
                    if gname != "g":
                        woff = gate_idx * 512 + jl * 128
                        for kp in range(2):
                            for q in range(4):
                                cols = slice(q * 512, (q + 1) * 512)
                                nc.tensor.matmul(
                                    ps[:, cols],
                                    wt8_sb[:, 2 * kp : 2 * kp + 2, woff : woff + 128],
                                    zt8[:, 2 * kp : 2 * kp + 2, cols],
                                    start=(kp == 0),
                                    stop=(kp == 1),
                                    perf_mode=DR,
                                )
                        func, scale = AF.Sigmoid, 1.0 / WSCALE
                    else:
                        for ks in range(4):
                            for q in range(4):
                                cols = slice(q * 512, (q + 1) * 512)
                                nc.tensor.matmul(
                                    ps[:, cols],
                                    wtg_sb[:, ks, jl * 128 : (jl + 1) * 128],
                                    ztb[:, ks, cols],
                                    start=(ks == 0),
                                    stop=(ks == 3),
                                )
                        func, scale = AF.Tanh, 1.0
                    gt = pgate.tile(
                        [128, R],
                        BF16,
                        tag=gname,
                        bufs=3 if gname == "o" else 2,
                        name=f"{gname}{gi}",
                    )
                    nc.scalar.activation(
                        gt[:], ps[:], func, bias=bias_sb[:, J : J + 1], scale=scale
                    )
                    d[gname] = gt
                pend[gi] = d

    nc.compile()
    return nc


def pack_inputs(inputs, short_term_memory, Wf, bf, Wi, bi, Wg, bg, Wo, bo):
    import ml_dtypes

    bf16 = ml_dtypes.bfloat16
    e4 = ml_dtypes.float8_e4m3

    def pretranspose(a):
        # [B, 512] -> [core*blk, p, ks, b] with a[core*8192 + k*2048 + b,
        # ks*128 + p] at [core*4+k, p, ks, b]
        t = np.asarray(a, np.float32).astype(bf16)
        t = t.reshape(N_CORES, NBLK, R, 4, 128).transpose(0, 1, 4, 3, 2)
        return np.ascontiguousarray(t).reshape(N_CORES * NBLK, 128, 4, R)

    x = pretranspose(inputs)
    s = pretranspose(short_term_memory)
    # fp8 gates f,i,o: wt8[p, ks, gi*512+j] = W[j, ks*128+p] * 4096
    wt8 = np.empty((128, 4, 3 * 512), e4)
    for gidx, W in enumerate([Wf, Wi, Wo]):
        Wt = np.asarray(W, np.float32).T  # [k, j]
        t = np.clip(Wt.reshape(4, 128, 512) * WSCALE, -240.0, 240.0)
        wt8[:, :, gidx * 512 : (gidx + 1) * 512] = t.transpose(1, 0, 2).astype(e4)
    Wgt = np.asarray(Wg, np.float32).T
    wtg = np.ascontiguousarray(
        Wgt.reshape(4, 128, 512).transpose(1, 0, 2).astype(bf16)
    )
    # bias[p, J]: J = gate_idx*4 + jl (gate order f,i,o,g); value b[jl*128+p]
    bias = np.empty((128, 16), np.float32)
    for gidx, b in enumerate([bf, bi, bo, bg]):
        bias[:, gidx * 4 : (gidx + 1) * 4] = (
            np.asarray(b, np.float32).reshape(4, 128).T
        )
    return {"x": x, "s": s, "wt8": wt8, "wtg": wtg, "bias": bias}


class Runner:
    """Compiles the module once and keeps a reusable jitted executor."""

    def __init__(self, nc=None, n_cores=N_CORES):
        import jax
        from concourse import bass2jax as b2j

        self.jax = jax
        self.n_cores = n_cores
        self.nc = nc or build_module(n_cores=n_cores)
        b2j.install_neuronx_cc_hook()

        # dump the final (renamed) NEFF so neuron-profile can pair it with NTFFs
        if not getattr(b2j, "_neff_dump_patched", False):
            orig = b2j.rename_neff_tensors_and_patch_header

            def _patched(neff_path, mapping):
                data = orig(neff_path, mapping)
                with open(NEFF_DUMP, "wb") as f:
                    f.write(data)
                return data

            b2j.rename_neff_tensors_and_patch_header = _patched
            b2j._neff_dump_patched = True

        from jax.experimental.shard_map import shard_map
        from jax.sharding import Mesh, NamedSharding, PartitionSpec

        part_name = (
            self.nc.partition_id_tensor.name if self.nc.partition_id_tensor else None
        )
        in_names, out_names, out_avals = [], [], []
        self.out_shapes = {}
        for alloc in self.nc.m.functions[0].allocations:
            if not isinstance(alloc, mybir.MemoryLocationSet):
                continue
            name = alloc.memorylocations[0].name
            if alloc.kind == "ExternalInput":
                if name != part_name:
                    in_names.append(name)
            elif alloc.kind == "ExternalOutput":
                out_names.append(name)
                shape = tuple(alloc.tensor_shape)
                dt = mybir.dt.np(alloc.dtype)
                out_avals.append(jax.core.ShapedArray(shape, dt))
                self.out_shapes[name] = (shape, dt)
        self.in_names, self.out_names = in_names, out_names
        nc_ref = self.nc

        bind_names = list(in_names) + list(out_names)
        if part_name is not None:
            bind_names.append(part_name)

        def _body(*args):
            operands = list(args)
            if part_name is not None:
                operands.append(b2j.partition_id_tensor())
            outs = b2j._bass_exec_p.bind(
                *operands,
                out_avals=tuple(out_avals),
                in_names=tuple(bind_names),
                out_names=tuple(out_names),
                lowering_input_output_aliases=(),
                sim_require_finite=False,
                sim_require_nnan=False,
                nc=nc_ref,
            )
            return tuple(outs)

        devices = jax.devices()[: self.n_cores]
        mesh = Mesh(np.asarray(devices), ("core",))
        spec = PartitionSpec("core")
        n_args = len(in_names) + len(out_names)
        self.sharding = NamedSharding(mesh, spec)
        self.fn = jax.jit(
            shard_map(
                _body,
                mesh=mesh,
                in_specs=(spec,) * n_args,
                out_specs=(spec,) * len(out_names),
                check_rep=False,
            ),
            keep_unused=True,
        )
        self._dev_args = None

    def stage(self, packed):
        """Transfer inputs (sharded/replicated as needed) to devices once."""
        jax = self.jax
        nc_n = self.n_cores
        args = []
        for name in self.in_names:
            a = packed[name]
            if name in ("x", "s"):
                glob = a  # [B, H]; axis-0 shard = per-core [B_CORE, H]
            else:
                glob = np.concatenate([a] * nc_n, axis=0)  # replicate
            args.append(glob)
        for name in self.out_names:
            shape, dt = self.out_shapes[name]
            args.append(np.zeros((shape[0] * nc_n,) + shape[1:], dt))
        self._dev_args = [jax.device_put(a, self.sharding) for a in args]

    def execute(self):
        outs = self.fn(*self._dev_args)
        self.jax.block_until_ready(outs)
        return outs

    def run(self, packed):
        self.stage(packed)
        outs = self.execute()
        res = {}
        for name, arr in zip(self.out_names, outs):
            a = np.asarray(arr)  # [n_cores*d0, ...]
            shape, _ = self.out_shapes[name]
            res[name] = a.reshape((self.n_cores, shape[0]) + tuple(shape[1:]))
        return res


_RUNNER = None


def _get_runner():
    global _RUNNER
    if _RUNNER is None:
        _RUNNER = Runner()
    return _RUNNER


def kernel(**inputs):
    r = _get_runner()
    packed = pack_inputs(**inputs)
    res = r.run(packed)
    per_core = res["out"]  # [8, 512 j, 2, 8192 b] bf16
    o32 = per_core.astype(np.float32)
    # final[ch, core*8192 + b, j] = o32[core, j, ch, b]
    return np.ascontiguousarray(
        o32.transpose(2, 0, 3, 1).reshape(2, B, H)
    )


if __name__ == "__main__":
    nc = build_module()
    print("module built + compiled OK")
